# revision 14
# baseline (speedup 1.0000x reference)
"""AttnDecoderRNN single-step kernel for 8 Trainium2 NeuronCores.

Strategy (tensor-parallel on vocab V, per sharding hint):
  - The dominant cost is the output projection [B,2H] @ W_out.T with
    W_out [50257, 1024] (~206 MB, memory-bound).  W_out.T is sharded
    column-wise across 8 cores (6288 padded columns each).
  - Attention + LSTM state is small and replicated on every core.
  - Launch 1 (per core): attention -> LSTM cell -> logits shard with
    fused bias, plus online-softmax stats (row max m_i, sum exp s_i).
  - Host combines the 8 (m_i, s_i) pairs into a global logsumexp
    (256 scalars of glue math).
  - Launch 2 (per core): subtract lse from a batch-shard of the logits
    on device -> log-softmax output.

Host-side prep is limited to sharding/staging: transposing weights into
the layout the TensorEngine needs (it contracts over the partition dim),
padding V to a multiple of 8, and gathering the 32 embedding rows.
"""

import os
import sys

import numpy as np

for _p in ("/opt/trn_rl_repo",):
    if os.path.isdir(_p) and _p not in sys.path:
        sys.path.insert(0, _p)

import concourse.bass as bass  # noqa: E402
from concourse import bacc  # noqa: E402
import concourse.mybir as mybir  # noqa: E402
import concourse.tile as tile  # noqa: E402
from concourse.masks import make_identity  # noqa: E402

F32 = mybir.dt.float32
AX = mybir.AxisListType
OP = mybir.AluOpType
AF = mybir.ActivationFunctionType

H = 512
V = 50257
B = 32
L = 50
NCORES = 8
VP = 6288           # per-core padded vocab shard; 8*6288 = 50304 >= 50257
VPAD = NCORES * VP  # 50304
NEG = -1.0e30       # bias for padded vocab columns

# logits chunk widths for the output projection (PSUM free dim <= 512 f32)
CHUNKS = [512] * 12 + [144]
CSTART = [sum(CHUNKS[:i]) for i in range(len(CHUNKS))]
# chunk groups sharing one set of W k-tiles in SBUF
GROUPS = [(0, 1), (2, 3), (4, 5), (6, 7), (8, 9), (10, 11), (12,)]

# launch 2 layout: full padded logits resharded by batch, 4 rows/core
RB = B // NCORES            # 4
W2 = VPAD // 32             # 1572  (each row split into 32 partition chunks)


def _load_hc(nc, out_tile, src2d):
    """DMA [32, 512] DRAM-view data into [128, 128] (hc,b)-packed SBUF tile:
    out[(hc*32)+b, hs] = src[b, hc*128+hs]."""
    for hc in range(4):
        nc.sync.dma_start(
            out=out_tile[hc * 32 : (hc + 1) * 32, :],
            in_=src2d[:, hc * 128 : (hc + 1) * 128],
        )


def build_main():
    nc = bacc.Bacc()
    enc = nc.dram_tensor("enc", [L, B, H], F32, kind="ExternalInput")
    h0 = nc.dram_tensor("h0in", [B, H], F32, kind="ExternalInput")
    c0 = nc.dram_tensor("c0in", [B, H], F32, kind="ExternalInput")
    embr = nc.dram_tensor("embr", [B, H], F32, kind="ExternalInput")
    wcat = nc.dram_tensor("wcat", [3 * H, 4 * H], F32, kind="ExternalInput")
    gbias = nc.dram_tensor("gbias", [1, 4 * H], F32, kind="ExternalInput")
    wout = nc.dram_tensor("wout", [2 * H, VP], F32, kind="ExternalInput")
    bout = nc.dram_tensor("bout", [1, VP], F32, kind="ExternalInput")

    logits = nc.dram_tensor("logits", [B, VP], F32, kind="ExternalOutput")
    mstat = nc.dram_tensor("mstat", [B, 1], F32, kind="ExternalOutput")
    sstat = nc.dram_tensor("sstat", [B, 1], F32, kind="ExternalOutput")
    h1o = nc.dram_tensor("h1o", [B, H], F32, kind="ExternalOutput")
    c1o = nc.dram_tensor("c1o", [B, H], F32, kind="ExternalOutput")
    attno = nc.dram_tensor("attno", [B, L], F32, kind="ExternalOutput")

    from contextlib import ExitStack

    with tile.TileContext(nc) as tc, ExitStack() as ctx:
        consts = ctx.enter_context(tc.tile_pool(name="consts", bufs=1))
        apool = ctx.enter_context(tc.tile_pool(name="attn", bufs=1))
        small = ctx.enter_context(tc.tile_pool(name="small", bufs=1))
        lstm = ctx.enter_context(tc.tile_pool(name="lstm", bufs=1))
        wcpool = ctx.enter_context(tc.tile_pool(name="wcat", bufs=3))
        wopool = ctx.enter_context(tc.tile_pool(name="wout", bufs=10))
        stage = ctx.enter_context(tc.tile_pool(name="stage", bufs=3))
        escr = ctx.enter_context(tc.tile_pool(name="escr", bufs=2))
        psum_t = ctx.enter_context(tc.tile_pool(name="psum_t", bufs=1, space="PSUM"))
        psum_g = ctx.enter_context(tc.tile_pool(name="psum_g", bufs=1, space="PSUM"))
        psum_l = ctx.enter_context(tc.tile_pool(name="psum_l", bufs=2, space="PSUM"))

        # ---- constants ----
        id128 = consts.tile([128, 32], F32)  # 4x stacked 32x32 identity
        for j in range(4):
            make_identity(nc, id128[32 * j : 32 * (j + 1), :])
        ones = consts.tile([1, B], F32)
        nc.vector.memset(ones, 1.0)

        # ---- load small operands ----
        # (hc,b)-packed layouts: partition p = hc*32+b, free = 128 h's of chunk hc
        h0r = apool.tile([128, 128], F32)
        _load_hc(nc, h0r, h0[:, :])
        enc_t = apool.tile([128, L, 128], F32)
        for hc in range(4):
            nc.sync.dma_start(
                out=enc_t[hc * 32 : (hc + 1) * 32, :, :],
                in_=enc[:, :, hc * 128 : (hc + 1) * 128].rearrange("l b h -> b l h"),
            )
        emb_sb = small.tile([B, H], F32)
        nc.sync.dma_start(out=emb_sb, in_=embr[:, :])
        c0_sb = small.tile([B, H], F32)
        nc.sync.dma_start(out=c0_sb, in_=c0[:, :])
        gb_sb = small.tile([1, 4 * H], F32)
        nc.sync.dma_start(out=gb_sb, in_=gbias[:, :])
        bout_sb = small.tile([1, VP], F32)
        nc.sync.dma_start(out=bout_sb, in_=bout[:, :])

        # ---- attention energies: e[b,l] = sum_h h0[b,h] enc[l,b,h] ----
        prod = apool.tile([128, L, 128], F32)
        for hc in range(4):
            sl = slice(hc * 32, (hc + 1) * 32)
            nc.vector.tensor_tensor(
                prod[sl, :, :], enc_t[sl, :, :],
                h0r[sl, None, :].to_broadcast((32, L, 128)), OP.mult,
            )
        e4 = apool.tile([128, L], F32)
        nc.vector.tensor_reduce(out=e4, in_=prod[:, :, :], axis=AX.X, op=OP.add)
        # fold the 4 h-chunk partition groups: e[b,l] = sum_hc e4[hc*32+b, l]
        # (id128 is 4 stacked 32x32 identities, so id128.T @ e4 does the fold)
        etile = psum_t.tile([B, L], F32, tag="pse")
        nc.tensor.matmul(etile, id128[:, :], e4[:, :], start=True, stop=True)

        # ---- softmax over l ----
        negmax = apool.tile([B, 1], F32)
        nc.vector.tensor_reduce(out=negmax, in_=etile, axis=AX.X, op=OP.max, negate=True)
        ssum = apool.tile([B, 1], F32)
        attn_rep = apool.tile([128, L], F32)
        nc.scalar.activation(
            out=attn_rep[0:32, :], in_=etile, func=AF.Exp, bias=negmax, scale=1.0,
            accum_out=ssum,
        )
        rsum = apool.tile([B, 1], F32)
        nc.vector.reciprocal(rsum, ssum)
        nc.vector.tensor_scalar_mul(attn_rep[0:32, :], attn_rep[0:32, :], rsum)
        nc.sync.dma_start(out=attno[:, :], in_=attn_rep[0:32, :])
        for j in range(1, 4):
            nc.sync.dma_start(
                out=attn_rep[32 * j : 32 * (j + 1), :], in_=attn_rep[0:32, :]
            )

        # ---- context: ctx[b,h] = sum_l attn[b,l] enc[l,b,h]  ((hc,b) layout) ----
        prod2 = apool.tile([128, L, 128], F32, tag="prod")  # reuse prod slot
        for hc in range(4):
            sl = slice(hc * 32, (hc + 1) * 32)
            nc.vector.tensor_tensor(
                prod2[sl, :, :], enc_t[sl, :, :],
                attn_rep[sl, :, None].to_broadcast((32, L, 128)), OP.mult,
            )
        ctx = apool.tile([128, 128], F32)
        nc.vector.tensor_reduce(
            out=ctx, in_=prod2.rearrange("p l h -> p h l"), axis=AX.X, op=OP.add
        )

        # ---- transpose x_cat = [emb, ctx, h0] into [128k, 32b] tiles ----
        xT = small.tile([128, 12, B], F32)
        for j in range(12):
            hc = j % 4
            if j < 4:
                src = emb_sb[:, hc * 128 : (hc + 1) * 128]
                tp = (0, 0)
            elif j < 8:
                src = ctx[hc * 32 : (hc + 1) * 32, :]
                tp = (32 * hc, 0)
            else:
                src = h0r[hc * 32 : (hc + 1) * 32, :]
                tp = (32 * hc, 0)
            pst = psum_t.tile([128, B], F32, tag="pst")
            nc.tensor.transpose(
                pst, src, id128[tp[0] : tp[0] + 32, :], tile_position=tp
            )
            nc.vector.tensor_copy(out=xT[:, j, :], in_=pst)

        # ---- LSTM gates: x_cat @ W_cat.T + bias, accumulated in PSUM ----
        gps = [psum_g.tile([B, 512], F32, tag=f"g{g}", name=f"g{g}") for g in range(4)]
        for k in range(12):
            wct = wcpool.tile([128, 4 * H], F32, tag="wcat")
            nc.sync.dma_start(out=wct, in_=wcat[k * 128 : (k + 1) * 128, :])
            for g in range(4):
                nc.tensor.matmul(
                    gps[g], xT[:, k, :], wct[:, g * 512 : (g + 1) * 512],
                    start=(k == 0), stop=False,
                )
        for g in range(4):
            nc.tensor.matmul(
                gps[g], ones[0:1, :], gb_sb[0:1, g * 512 : (g + 1) * 512],
                start=False, stop=True,
            )

        # ---- LSTM cell elementwise ----
        si = lstm.tile([B, 512], F32, tag="si")
        sf = lstm.tile([B, 512], F32, tag="sf")
        tg = lstm.tile([B, 512], F32, tag="tg")
        so = lstm.tile([B, 512], F32, tag="so")
        nc.scalar.activation(out=si, in_=gps[0], func=AF.Sigmoid)
        nc.scalar.activation(out=sf, in_=gps[1], func=AF.Sigmoid)
        nc.scalar.activation(out=tg, in_=gps[2], func=AF.Tanh)
        nc.scalar.activation(out=so, in_=gps[3], func=AF.Sigmoid)
        c1t = lstm.tile([B, 512], F32, tag="c1t")
        tmp = lstm.tile([B, 512], F32, tag="tmp")
        nc.vector.tensor_tensor(c1t, sf, c0_sb, OP.mult)
        nc.vector.tensor_tensor(tmp, si, tg, OP.mult)
        nc.vector.tensor_tensor(c1t, c1t, tmp, OP.add)
        tc1 = lstm.tile([B, 512], F32, tag="tc1")
        nc.scalar.activation(out=tc1, in_=c1t, func=AF.Tanh)
        h1t = lstm.tile([B, 512], F32, tag="h1t")
        nc.vector.tensor_tensor(h1t, so, tc1, OP.mult)
        nc.sync.dma_start(out=h1o[:, :], in_=h1t)
        nc.sync.dma_start(out=c1o[:, :], in_=c1t)

        # ---- transpose h1 into [128k, 32b] tiles (ctx tiles reused from xT) ----
        rnnT = small.tile([128, 4, B], F32)
        for hc in range(4):
            pst = psum_t.tile([128, B], F32, tag="pst")
            nc.tensor.transpose(pst, h1t[:, hc * 128 : (hc + 1) * 128], id128[0:32, :])
            nc.vector.tensor_copy(out=rnnT[:, hc, :], in_=pst)

        # ---- output projection + online softmax stats ----
        nmx = small.tile([B, len(CHUNKS)], F32)   # per-chunk -max
        sv = small.tile([B, len(CHUNKS)], F32)    # per-chunk sum exp(x - max_c)
        for chunk_ids in GROUPS:
            gs = CSTART[chunk_ids[0]]
            gw = sum(CHUNKS[c] for c in chunk_ids)
            wts = []
            for k in range(8):
                wt = wopool.tile([128, 1024], F32, tag="wout")
                nc.sync.dma_start(
                    out=wt[:, :gw], in_=wout[k * 128 : (k + 1) * 128, gs : gs + gw]
                )
                wts.append(wt)
            for c in chunk_ids:
                coff = CSTART[c] - gs
                cw = CHUNKS[c]
                ps = psum_l.tile([B, 512], F32, tag="pslog", name=f"pslog{c}")[:, :cw]
                for k in range(8):
                    lhs = rnnT[:, k, :] if k < 4 else xT[:, 4 + (k - 4), :]
                    nc.tensor.matmul(
                        ps, lhs, wts[k][:, coff : coff + cw],
                        start=(k == 0), stop=False,
                    )
                nc.tensor.matmul(
                    ps, ones[0:1, :], bout_sb[0:1, CSTART[c] : CSTART[c] + cw],
                    start=False, stop=True,
                )
                st = stage.tile([B, 512], F32, tag="stage", name=f"stage{c}")[:, :cw]
                nc.vector.tensor_copy(out=st, in_=ps)
                nc.vector.tensor_reduce(
                    out=nmx[:, c : c + 1], in_=ps, axis=AX.X, op=OP.max, negate=True
                )
                esc = escr.tile([B, 512], F32, tag="esc", name=f"esc{c}")[:, :cw]
                nc.scalar.activation(
                    out=esc, in_=ps, func=AF.Exp, bias=nmx[:, c : c + 1], scale=1.0,
                    accum_out=sv[:, c : c + 1],
                )
                nc.sync.dma_start(
                    out=logits[:, CSTART[c] : CSTART[c] + cw], in_=st
                )

        # ---- combine per-chunk stats: m = max_c mx_c, s = sum_c s_c exp(mx_c-m)
        negm = small.tile([B, 1], F32)
        nc.vector.tensor_reduce(out=negm, in_=nmx, axis=AX.X, op=OP.min)  # = -m
        expd = small.tile([B, len(CHUNKS)], F32)
        nc.scalar.activation(out=expd, in_=nmx, func=AF.Exp, bias=negm, scale=-1.0)
        scr13 = small.tile([B, len(CHUNKS)], F32)
        stot = small.tile([B, 1], F32)
        nc.vector.tensor_tensor(scr13, sv, expd, OP.mult)
        nc.vector.tensor_reduce(out=stot, in_=scr13, axis=AX.X, op=OP.add)
        msb = small.tile([B, 1], F32)
        nc.scalar.mul(out=msb, in_=negm, mul=-1.0)
        nc.sync.dma_start(out=mstat[:, :], in_=msb)
        nc.sync.dma_start(out=sstat[:, :], in_=stot)

    nc.compile()
    return nc


def build_fix():
    nc = bacc.Bacc()
    lgin = nc.dram_tensor("lgin", [RB, VPAD], F32, kind="ExternalInput")
    lserep = nc.dram_tensor("lserep", [128, 1], F32, kind="ExternalInput")
    lgout = nc.dram_tensor("lgout", [RB, VPAD], F32, kind="ExternalOutput")
    with tile.TileContext(nc) as tc:
        with tc.tile_pool(name="p", bufs=1) as pool:
            t = pool.tile([128, W2], F32)
            for b in range(RB):
                nc.sync.dma_start(
                    out=t[b * 32 : (b + 1) * 32, :],
                    in_=lgin[b : b + 1, :].rearrange("o (c w) -> (o c) w", c=32),
                )
            lse_sb = pool.tile([128, 1], F32)
            nc.sync.dma_start(out=lse_sb, in_=lserep[:, :])
            for b in range(RB):
                sl = slice(b * 32, (b + 1) * 32)
                nc.vector.tensor_scalar_sub(
                    out=t[sl, :], in0=t[sl, :], scalar1=lse_sb[sl, :])
            for b in range(RB):
                nc.sync.dma_start(
                    out=lgout[b : b + 1, :].rearrange("o (c w) -> (o c) w", c=32),
                    in_=t[b * 32 : (b + 1) * 32, :],
                )
    nc.compile()
    return nc


# ---------------------------------------------------------------------------
# host side
# ---------------------------------------------------------------------------

_progs = {}
_prep_cache = {}
LAST_RESULTS = []  # BassKernelResults of the most recent kernel() call


def _get_prog(name):
    if name not in _progs:
        _progs[name] = {"main": build_main, "fix": build_fix}[name]()
    return _progs[name]


def _fingerprint(arr):
    a = np.asarray(arr)
    s = a.reshape(-1)
    step = max(1, s.size // 64)
    return (a.shape, str(a.dtype), s[::step][:64].tobytes())


def _prep_weights(W_ih, W_hh, b_ih, b_hh, W_out, b_out):
    key = (_fingerprint(W_out), _fingerprint(W_ih), _fingerprint(W_hh))
    hit = _prep_cache.get("w")
    if hit is not None and hit[0] == key:
        return hit[1]
    W_ih = np.asarray(W_ih, np.float32)
    W_hh = np.asarray(W_hh, np.float32)
    W_out = np.asarray(W_out, np.float32)
    wcat = np.ascontiguousarray(np.concatenate([W_ih, W_hh], axis=1).T)  # [1536,2048]
    gbias = (np.asarray(b_ih, np.float32) + np.asarray(b_hh, np.float32))[None, :]
    wt = np.zeros((2 * H, VPAD), np.float32)
    wt[:, :V] = W_out.T
    bp = np.full((VPAD,), NEG, np.float32)
    bp[:V] = np.asarray(b_out, np.float32)
    wshards = [np.ascontiguousarray(wt[:, c * VP : (c + 1) * VP]) for c in range(NCORES)]
    bshards = [np.ascontiguousarray(bp[c * VP : (c + 1) * VP])[None, :] for c in range(NCORES)]
    out = (wcat, gbias, wshards, bshards)
    _prep_cache["w"] = (key, out)
    return out


def _install_ntff_hook_shim():
    """Provide antenv.axon_hooks when the image lacks it, so
    run_bass_kernel_spmd(trace=True) can capture NTFF profiles via the
    axon .so's C ABI (mirrors trn_agent_boot's _ntff_profile_via_ctypes)."""
    try:
        import antenv.axon_hooks  # noqa: F401
        return
    except ImportError:
        pass
    import contextlib
    import ctypes
    import types

    import antenv

    so_path = "/opt/axon/libaxon_pjrt.so"
    hook = None
    if os.path.exists(so_path):
        lib = ctypes.CDLL(so_path)
        if hasattr(lib, "axon_start_nrt_profile"):
            lib.axon_start_nrt_profile.argtypes = [
                ctypes.POINTER(ctypes.c_int64), ctypes.c_size_t]
            lib.axon_start_nrt_profile.restype = ctypes.c_int64
            lib.axon_stop_nrt_profile.argtypes = [ctypes.c_char_p]
            lib.axon_stop_nrt_profile.restype = ctypes.c_int64

            @contextlib.contextmanager
            def _hook(output_dir, device_ids):
                import jax
                jax.devices()
                if device_ids:
                    ids = (ctypes.c_int64 * len(device_ids))(*device_ids)
                    rc = lib.axon_start_nrt_profile(ids, len(device_ids))
                else:
                    rc = lib.axon_start_nrt_profile(None, 0)
                if rc != 0:
                    raise RuntimeError(f"axon_start_nrt_profile rc={rc}")
                try:
                    yield
                finally:
                    n = lib.axon_stop_nrt_profile(str(output_dir).encode())
                    print(f"ntff profile: {n} file(s) -> {output_dir}",
                          file=sys.stderr)

            hook = _hook

    mod = types.ModuleType("antenv.axon_hooks")
    mod._hook = hook
    mod.get_axon_ntff_profile_hook = lambda: mod._hook
    mod.set_axon_ntff_profile_hook = lambda h: setattr(mod, "_hook", h)
    sys.modules["antenv.axon_hooks"] = mod
    antenv.axon_hooks = mod


def kernel(input_data, h0, c0, encoder_outputs, emb, W_ih, W_hh, b_ih, b_hh,
           W_out, b_out):
    from concourse.bass_utils import run_bass_kernel_spmd

    global LAST_RESULTS
    LAST_RESULTS = []
    trace = bool(int(os.environ.get("KERNEL_TRACE", "0")))
    if trace:
        _install_ntff_hook_shim()

    idx = np.asarray(input_data).astype(np.int64).reshape(-1)
    h0 = np.asarray(h0, np.float32).reshape(B, H)
    c0 = np.asarray(c0, np.float32).reshape(B, H)
    enc = np.ascontiguousarray(np.asarray(encoder_outputs, np.float32))
    embr = np.ascontiguousarray(np.asarray(emb, np.float32)[idx])  # [B, H]
    wcat, gbias, wshards, bshards = _prep_weights(W_ih, W_hh, b_ih, b_hh, W_out, b_out)

    in_maps = []
    for cidx in range(NCORES):
        in_maps.append({
            "enc": enc, "h0in": h0, "c0in": c0, "embr": embr,
            "wcat": wcat, "gbias": gbias,
            "wout": wshards[cidx], "bout": bshards[cidx],
        })
    res1 = run_bass_kernel_spmd(
        _get_prog("main"), in_maps, core_ids=list(range(NCORES)), trace=trace,
    )
    LAST_RESULTS.append(res1)
    r = res1.results

    # host glue: combine per-core logsumexp stats (8*32 scalars)
    ms = np.stack([r[c]["mstat"][:, 0] for c in range(NCORES)])  # [8, B]
    ss = np.stack([r[c]["sstat"][:, 0] for c in range(NCORES)])  # [8, B]
    m = ms.max(axis=0)
    s = (ss.astype(np.float64) * np.exp(ms.astype(np.float64) - m)).sum(axis=0)
    lse = (m + np.log(s)).astype(np.float32)  # [B]

    logits_full = np.concatenate([r[c]["logits"] for c in range(NCORES)], axis=1)

    in_maps2 = []
    for cidx in range(NCORES):
        rows = logits_full[cidx * RB : (cidx + 1) * RB]
        lr = np.repeat(lse[cidx * RB : (cidx + 1) * RB], 32)[:, None]
        in_maps2.append({
            "lgin": np.ascontiguousarray(rows),
            "lserep": np.ascontiguousarray(lr.astype(np.float32)),
        })
    res2 = run_bass_kernel_spmd(
        _get_prog("fix"), in_maps2, core_ids=list(range(NCORES)), trace=trace,
    )
    LAST_RESULTS.append(res2)
    logp = np.concatenate([res2.results[c]["lgout"] for c in range(NCORES)], axis=0)
    logp = np.ascontiguousarray(logp[:, :V])

    h1 = r[0]["h1o"][None]           # [1, B, H]
    c1 = r[0]["c1o"][None]
    attn = r[0]["attno"][:, None, :]  # [B, 1, L]
    return (logp, h1, c1, attn)


# ---------------------------------------------------------------------------
# numpy shard reference + CoreSim selftest (dev only; not used by harness)
# ---------------------------------------------------------------------------

def _np_shard_ref(core, enc, h0, c0, embr, wcat, gbias, wsh, bsh):
    e = np.einsum("bh,lbh->bl", h0, enc)
    ex = np.exp(e - e.max(1, keepdims=True))
    attn = ex / ex.sum(1, keepdims=True)
    ctx = np.einsum("bl,lbh->bh", attn, enc)
    xcat = np.concatenate([embr, ctx, h0], axis=1)
    gates = xcat @ wcat + gbias
    i_g, f_g, g_g, o_g = np.split(gates, 4, axis=1)
    sig = lambda x: 1.0 / (1.0 + np.exp(-x))
    c1 = sig(f_g) * c0 + sig(i_g) * np.tanh(g_g)
    h1 = sig(o_g) * np.tanh(c1)
    rnn = np.concatenate([h1, ctx], axis=1)
    lg = rnn @ wsh + bsh
    m = lg.max(1)
    s = np.exp(lg - m[:, None]).sum(1)
    return dict(attn=attn, h1=h1, c1=c1, logits=lg, m=m, s=s)


def _selftest_sim():
    from concourse.bass_interp import CoreSim

    rng = np.random.default_rng(0)
    h0 = rng.standard_normal((B, H), np.float32)
    c0 = rng.standard_normal((B, H), np.float32)
    enc = rng.standard_normal((L, B, H), np.float32)
    embr = rng.standard_normal((B, H), np.float32) * 0.02
    wcat, gbias, wshards, bshards = _prep_weights(
        rng.standard_normal((4 * H, 2 * H), np.float32) * 0.02,
        rng.standard_normal((4 * H, H), np.float32) * 0.02,
        rng.standard_normal((4 * H,), np.float32) * 0.01,
        rng.standard_normal((4 * H,), np.float32) * 0.01,
        rng.standard_normal((V, 2 * H), np.float32) * 0.02,
        rng.standard_normal((V,), np.float32) * 0.01)

    nc = _get_prog("main")
    for core in (0, NCORES - 1):
        sim = CoreSim(nc, trace=False)
        for name, arr in [("enc", enc), ("h0in", h0), ("c0in", c0),
                          ("embr", embr), ("wcat", wcat), ("gbias", gbias),
                          ("wout", wshards[core]), ("bout", bshards[core])]:
            sim.tensor(name)[:] = arr
        sim.simulate(check_with_hw=False)
        ref = _np_shard_ref(core, enc, h0, c0, embr, wcat, gbias,
                            wshards[core], bshards[core])
        for nm, got, want in [
            ("attn", sim.tensor("attno"), ref["attn"]),
            ("h1", sim.tensor("h1o"), ref["h1"]),
            ("c1", sim.tensor("c1o"), ref["c1"]),
            ("logits", sim.tensor("logits"), ref["logits"]),
            ("m", sim.tensor("mstat")[:, 0], ref["m"]),
            ("s", sim.tensor("sstat")[:, 0], ref["s"]),
        ]:
            got = np.asarray(got)
            err = np.abs(got - want).max() / (np.abs(want).max() + 1e-9)
            print(f"core {core} {nm}: relerr {err:.3e}")
            assert err < 2e-5, (core, nm, err)
    print("sim selftest main PASSED")

    # fix kernel
    ncf = _get_prog("fix")
    sim = CoreSim(ncf, trace=False)
    lg = np.random.randn(RB, VPAD).astype(np.float32)
    lse = np.random.randn(RB).astype(np.float32)
    sim.tensor("lgin")[:] = lg
    sim.tensor("lserep")[:] = np.repeat(lse, 32)[:, None]
    sim.simulate(check_with_hw=False)
    want = lg - lse[:, None]
    err = np.abs(sim.tensor("lgout") - want).max()
    print(f"fix kernel abserr {err:.3e}")
    assert err < 1e-6
    print("sim selftest fix PASSED")


def _make_ref_inputs():
    import jax
    import jax.numpy as jnp
    key = jax.random.key(0)
    ks = jax.random.split(key, 12)
    return {
        "input_data": np.asarray(jax.random.randint(ks[0], (1, B), 0, V)),
        "h0": np.asarray(jax.random.normal(ks[1], (1, B, H), jnp.float32)),
        "c0": np.asarray(jax.random.normal(ks[2], (1, B, H), jnp.float32)),
        "encoder_outputs": np.asarray(jax.random.normal(ks[3], (L, B, H), jnp.float32)),
        "emb": np.asarray(jax.random.normal(ks[4], (V, H), jnp.float32) * 0.02),
        "W_ih": np.asarray(jax.random.normal(ks[5], (4 * H, 2 * H), jnp.float32) * 0.02),
        "W_hh": np.asarray(jax.random.normal(ks[6], (4 * H, H), jnp.float32) * 0.02),
        "b_ih": np.zeros((4 * H,), np.float32),
        "b_hh": np.zeros((4 * H,), np.float32),
        "W_out": np.asarray(jax.random.normal(ks[7], (V, 2 * H), jnp.float32) * 0.02),
        "b_out": np.zeros((V,), np.float32),
    }


if __name__ == "__main__":
    if "--sim" in sys.argv:
        _selftest_sim()
